# revision 17
# baseline (speedup 1.0000x reference)
"""Trainium2 Bass kernel for nn_DavidBeansV2 (sparse wormhole attention).

Math (per batch item b, derived from the reference):
  xp = x[b, 1:, :]                                  # [P, D]
  q  = l2norm(xp @ Wq + bq); k = l2norm(xp @ Wk + bk)
  S  = q @ k.T + pos_bias    (diag forced very negative)
  topk16 per row of S/TEMP -> softmax weights w (zero elsewhere)
  v  = xp @ Wv + bv
  out[b] = (w / rowsum(w)) @ v                      # [P, D]
The multihead gather+combine with routes shared across heads is exactly a
row-sparse [P,P] x [P,D] matmul, so we compute it densely on the PE with a
masked-softmax weight matrix.

Sharding: data-parallel over batch B=8 across the 8 NeuronCores.

Precision: top-16 selection needs ~1e-6-accurate scores (the 16/17
boundary gaps concentrate near zero), which rules out any single-pass
matmul.  Instead of the 3x-cost compensated-fp16 scheme, each exact
matmul A@B runs as
    r12(A) @ r12(B)            one fp32r pass   (1 cyc/row, 12-bit operands)
  + [Al' | A'] @ [B' | Bl']    one fp8e5 DoubleRow pass (0.5 cyc/row)
where Al = A - r12(A) and the DoubleRow pair computes Al@B + A@Bl with
host/device-baked power-of-two scale splits so both fp8 products land at
natural scale and accumulate into the same PSUM group.  Total 1.5
cyc/row-equivalent vs 3.0 for the compensated path, with ~1e-6 score
error (fp8 only ever quantizes the ~2^-13-magnitude residual terms).
Normalization is factored out of q/k and applied to the scores
(S = (qraw.kraw) * rq[p] * rk[col] + pb) in fp32 vector ops, keeping all
matmul operands at their raw (losslessly split) values.  V projection is
a single fp32r pass (smooth error only); the combine runs in fp16.
"""

import numpy as np
import ml_dtypes

import concourse.mybir as mybir
import concourse.tile as tile
from concourse import bass_isa
from concourse import bacc
from concourse.bass_utils import run_bass_kernel_spmd
from concourse.masks import make_identity

F32 = mybir.dt.float32
F32R = mybir.dt.float32r
F16 = mybir.dt.float16
F8E5 = mybir.dt.float8e5
AF = mybir.ActivationFunctionType
OP = mybir.AluOpType
DR = mybir.MatmulPerfMode.DoubleRow
E5 = ml_dtypes.float8_e5m2

B, P, D = 8, 1024, 768
TEMP = 0.1
KC = D // 128     # 6 contraction chunks
PB = P // 128     # 8 row blocks
MINVAL = -50.0    # match_replace fill; below any real score, above diag fill
DIAGVAL = -10000.0
ASC = 2.0 ** 6    # xl cross scale: (xl*ASC) @ (Wr/ASC)
BSC = 2.0 ** 10   # Wl cross scale: (x/BSC) @ (Wl*BSC)


def build_program(with_bias: bool):
    nc = bacc.Bacc(
        "TRN2",
        target_bir_lowering=False,
        debug=False,
        enable_asserts=False,
        num_devices=B,
    )
    xr_d = nc.dram_tensor("xr", [D, P], F32R, kind="ExternalInput").ap()
    x8_d = nc.dram_tensor("x8", [D, 2, P], F8E5, kind="ExternalInput").ap()
    wqr_d = nc.dram_tensor("wqr", [D, D], F32R, kind="ExternalInput").ap()
    wq8_d = nc.dram_tensor("wq8", [D, 2, D], F8E5, kind="ExternalInput").ap()
    wkr_d = nc.dram_tensor("wkr", [D, D], F32R, kind="ExternalInput").ap()
    wk8_d = nc.dram_tensor("wk8", [D, 2, D], F8E5, kind="ExternalInput").ap()
    wvr_d = nc.dram_tensor("wvr", [D, D], F32R, kind="ExternalInput").ap()
    pb = nc.dram_tensor("pb", [P, P], F32, kind="ExternalInput").ap()
    if with_bias:
        bqkv = nc.dram_tensor("bqkv", [1, 3, D], F32, kind="ExternalInput").ap()
    out = nc.dram_tensor("out", [P, D], F16, kind="ExternalOutput").ap()

    with tile.TileContext(nc) as tc:
        consts = tc.alloc_tile_pool(name="consts", bufs=1)
        persist = tc.alloc_tile_pool(name="persist", bufs=1)
        wk_pool = tc.alloc_tile_pool(name="wk_pool", bufs=1)
        inp_pool = tc.alloc_tile_pool(name="inp", bufs=1)
        work2 = tc.alloc_tile_pool(name="work2", bufs=1)
        wq_pool = tc.alloc_tile_pool(name="wq_pool", bufs=1)
        psum2 = tc.alloc_tile_pool(name="psum2", bufs=1, space="PSUM")

        ident = consts.tile([128, 128], F16, tag="ident")
        make_identity(nc, ident)
        # warm-up matmuls: keep the PE busy through the initial input-DMA
        # wait so the pstate clock is at full rate when real work starts
        warm_ps = psum2.tile([1, 64], F32, tag="warm_ps")
        for _ in range(60):
            nc.tensor.matmul(warm_ps, ident[:, 0:1], ident[:, 0:64],
                             start=True, stop=True)
        ones_row = consts.tile([1, 512], F32, tag="ones_row")
        nc.vector.memset(ones_row, 1.0)

        # ---- load inputs, chunked so the first matmuls start early ----
        xr_sb = inp_pool.tile([128, KC, P], F32R, tag="xr_sb", name="xr_sb")
        x8_sb = inp_pool.tile([128, KC, 2, P], F8E5, tag="x8_sb", name="x8_sb")
        wqr_sb = wq_pool.tile([128, KC, D], F32R, tag="wqr_sb", name="wqr_sb")
        wq8_sb = wq_pool.tile([128, KC, 2, D], F8E5, tag="wq8_sb", name="wq8_sb")
        wkr_sb = wk_pool.tile([128, KC, D], F32R, tag="wkr_sb", name="wkr_sb")
        wk8_sb = wk_pool.tile([128, KC, 2, D], F8E5, tag="wk8_sb", name="wk8_sb")

        xr_src = xr_d.rearrange("(o p) f -> p o f", p=128)
        x8_src = x8_d.rearrange("(o p) t f -> p o t f", p=128)
        wqr_src = wqr_d.rearrange("(o p) f -> p o f", p=128)
        wq8_src = wq8_d.rearrange("(o p) t f -> p o t f", p=128)
        wkr_src = wkr_d.rearrange("(o p) f -> p o f", p=128)
        wk8_src = wk8_d.rearrange("(o p) t f -> p o t f", p=128)

        # half-granular x loads so the dc-major matmul sweeps chase arrivals
        h0, h1 = slice(0, 512), slice(512, P)
        for dc in range(KC):
            nc.sync.dma_start(xr_sb[:, dc, h0], xr_src[:, dc, h0])
            nc.sync.dma_start(wqr_sb[:, dc, :], wqr_src[:, dc, :])
        for dc in range(KC):
            nc.sync.dma_start(x8_sb[:, dc, :, h0], x8_src[:, dc, :, h0])
            nc.sync.dma_start(wq8_sb[:, dc, :, :], wq8_src[:, dc, :, :])
        for dc in range(KC):
            nc.sync.dma_start(xr_sb[:, dc, h1], xr_src[:, dc, h1])
        for dc in range(KC):
            nc.sync.dma_start(x8_sb[:, dc, :, h1], x8_src[:, dc, :, h1])
        for dc in range(KC):
            nc.sync.dma_start(wkr_sb[:, dc, :], wkr_src[:, dc, :])
        for dc in range(KC):
            nc.sync.dma_start(wk8_sb[:, dc, :, :], wk8_src[:, dc, :, :])
        if with_bias:
            bias_sb = consts.tile([1, 3, D], F32, tag="bias_sb")
            nc.sync.dma_start(bias_sb, bqkv)

        # persistent q/k operands for the scores stage
        q_r = persist.tile([128, KC, P], F32R, tag="q_r", name="q_r")
        k_r = persist.tile([128, KC, P], F32R, tag="k_r", name="k_r")
        q_8 = persist.tile([128, KC, 2, P], F8E5, tag="q_8", name="q_8")
        k_8 = persist.tile([128, KC, 2, P], F8E5, tag="k_8", name="k_8")
        v_sb = persist.tile([128, PB, D], F16, tag="v_sb")
        wpack = {"q": (wqr_sb, wq8_sb), "k": (wkr_sb, wk8_sb)}
        rpack = {"q": (q_r, q_8, 0, 1), "k": (k_r, k_8, 1, 0)}

        # ---- raw q/k projections (f32r + fp8 DoubleRow crosses) + norms ----
        rinv_rows = {}
        for ti, nm in enumerate(("q", "k")):
            wr, w8 = wpack[nm]
            t_r, t_8, l_slot, full_slot = rpack[nm]
            sq_acc = work2.tile([128, P], F32, tag="sq_acc")
            # dc-major sweeps over half-width PSUM groups (one bank per dblk)
            # so each arriving x/w chunk immediately unlocks work on all six
            # output blocks instead of serializing behind the full load.
            for sl in range(2):
                s = slice(sl * 512, (sl + 1) * 512)
                mm = [psum2.tile([128, 512], F32, tag=f"mmh{d}",
                                 name=f"mmh{d}", bufs=1) for d in range(KC)]
                for dc in range(KC):
                    for dblk in range(KC):
                        nc.tensor.matmul(
                            mm[dblk],
                            wr[:, dc, dblk * 128:(dblk + 1) * 128],
                            xr_sb[:, dc, s],
                            start=(dc == 0),
                            stop=False,
                        )
                for dc in range(KC):
                    for dblk in range(KC):
                        nc.tensor.matmul(
                            mm[dblk],
                            w8[:, dc, :, dblk * 128:(dblk + 1) * 128],
                            x8_sb[:, dc, :, s],
                            start=False,
                            stop=(dc == KC - 1) and not with_bias,
                            perf_mode=DR,
                        )
                if with_bias:
                    for dblk in range(KC):
                        nc.tensor.matmul(
                            mm[dblk],
                            bias_sb[:, ti, dblk * 128:(dblk + 1) * 128],
                            ones_row,
                            start=False,
                            stop=True,
                        )
                # split raw projection into f32r hi + fp8 residual/full pair;
                # consumers spread across act/DVE/Pool so no engine queue
                # backs up behind the PSUM rotations
                for dblk in range(KC):
                    nc.scalar.activation(t_r[:, dblk, s], mm[dblk], AF.Identity)
                    nc.scalar.activation(t_8[:, dblk, full_slot, s], mm[dblk],
                                         AF.Identity)
                    nc.vector.tensor_sub(t_8[:, dblk, l_slot, s], mm[dblk],
                                         t_r[:, dblk, s].bitcast(F32))
                    # norm^2 via q_raw*r12(q_raw): 1.3e-6 relative, row-uniform
                    if dblk == 0:
                        nc.vector.tensor_mul(sq_acc[:, s], mm[dblk],
                                             t_r[:, dblk, s].bitcast(F32))
                    else:
                        sq_sb = work2.tile([128, 512], F32, tag="sq_sb")
                        nc.vector.tensor_mul(sq_sb, mm[dblk],
                                             t_r[:, dblk, s].bitcast(F32))
                        nc.gpsimd.tensor_add(sq_acc[:, s], sq_acc[:, s], sq_sb)
            # norm2 = sum over partitions of sq_acc (gpsimd tree reduce),
            # broadcast to all partitions; row 0 feeds the rsqrt chain.
            allr = work2.tile([128, P], F32, tag="sq_sb")
            nc.gpsimd.partition_all_reduce(allr, sq_acc, channels=128,
                                           reduce_op=bass_isa.ReduceOp.add)
            norm2_sb = allr[0:1, :]
            # rinv = norm2^-0.5 via exp(-0.5*ln(norm2)) + one Newton step
            # (ACT spline budgets are too loose for the flip-sensitive top-16)
            ln_row = work2.tile([1, P], F32, tag="ln_row")
            nc.scalar.activation(ln_row, norm2_sb, AF.Ln)
            rinv_row = work2.tile([1, P], F32, tag=f"rinv_{nm}",
                                  name=f"rinv_{nm}")
            nc.scalar.activation(rinv_row, ln_row, AF.Exp, scale=-0.5)
            rr = work2.tile([1, P], F32, tag="rr")
            nc.vector.tensor_mul(rr, rinv_row, rinv_row)
            nc.vector.tensor_mul(rr, rr, norm2_sb)
            nc.vector.tensor_scalar(rr, rr, -0.5, 1.5, op0=OP.mult, op1=OP.add)
            nc.vector.tensor_mul(rinv_row, rinv_row, rr)
            rinv_rows[nm] = rinv_row
            if nm == "q":
                # Wv loads into the arena wq_pool is about to free
                wq_pool.release()
                wv_pool = tc.alloc_tile_pool(name="wv_pool", bufs=1)
                wvr_sb = wv_pool.tile([128, KC, D], F32R, tag="wvr_sb",
                                      name="wvr_sb")
                wvr_src = wvr_d.rearrange("(o p) f -> p o f", p=128)
                for dc in range(KC):
                    nc.sync.dma_start(wvr_sb[:, dc, :], wvr_src[:, dc, :])

        # ---- v projection (single f32r pass, natural [p, d] layout) ----
        for pblk in range(PB):
            vh = [psum2.tile([128, 512], F32, tag=f"mmh{(2 * pblk + i) % KC}",
                             name=f"mmh{(2 * pblk + i) % KC}", bufs=1)
                  for i in range(2)]
            for dc in range(KC):
                for sl, s, n in ((0, slice(0, 512), 512), (1, slice(512, D), 256)):
                    nc.tensor.matmul(
                        vh[sl][:, :n],
                        xr_sb[:, dc, pblk * 128:(pblk + 1) * 128],
                        wvr_sb[:, dc, s],
                        start=(dc == 0),
                        stop=(dc == KC - 1) and not with_bias,
                    )
            if with_bias:
                for sl, s, n in ((0, slice(0, 512), 512), (1, slice(512, D), 256)):
                    nc.tensor.matmul(
                        vh[sl][:, :n],
                        ones_row[:, :128],
                        bias_sb[:, 2, s],
                        start=False,
                        stop=True,
                    )
            nc.scalar.activation(v_sb[:, pblk, 0:512], vh[0], AF.Identity)
            nc.scalar.activation(v_sb[:, pblk, 512:D], vh[1][:, :256], AF.Identity)

        # ---- relayout rinv_q -> per-partition columns, rinv_k -> bcast ----
        # (fp32 matmuls: rinv values must not be re-rounded to f32r)
        rq_cols = persist.tile([128, PB], F32, tag="rq_cols")
        rq_ps = psum2.tile([128, PB], F32, tag="rq_ps", name="rq_ps", bufs=1)
        for j in range(PB):
            nc.tensor.matmul(
                rq_ps[:, j:j + 1],
                rinv_rows["q"][:, j * 128:(j + 1) * 128],
                ones_row[:, 0:1],
                start=True,
                stop=True,
            )
        nc.scalar.activation(rq_cols, rq_ps, AF.Identity)
        rk_bcast = persist.tile([128, P], F32, tag="rk_bcast")
        for sl in range(2):
            s = slice(sl * 512, (sl + 1) * 512)
            bc_ps = psum2.tile([128, 512], F32, tag=f"mmh{sl}",
                               name=f"mmh{sl}", bufs=1)
            nc.tensor.matmul(bc_ps, ones_row[:, :128],
                             rinv_rows["k"][:, s], start=True, stop=True)
            nc.scalar.activation(rk_bcast[:, s], bc_ps, AF.Identity)

        wv_pool.release()
        work2.release()
        inp_pool.release()
        wk_pool.release()
        psum2.release()

        # ---- per row-block: scores, top-16 softmax, combine ----
        work3 = tc.alloc_tile_pool(name="work3", bufs=2)
        wpool = tc.alloc_tile_pool(name="wpool", bufs=3)
        psum3 = tc.alloc_tile_pool(name="psum3", bufs=1, space="PSUM")

        def emit_scores(pblk):
            """S matmuls + vector chain through exp; returns (w_sb, rden)."""
            pbs = slice(pblk * 128, (pblk + 1) * 128)
            s_ps = psum3.tile([128, P], F32, tag="s_ps", name="s_ps", bufs=2)
            pb_sb = work3.tile([128, P], F32, tag="pb_sb")
            nc.sync.dma_start(pb_sb, pb[pbs, :])
            s_sb = work3.tile([128, P], F32, tag="s_sb")
            # compute each 512-half to completion so S = Sraw*rq*rk + pb
            # post-processing of half 0 overlaps half 1's matmuls
            for sl in range(2):
                s = slice(sl * 512, (sl + 1) * 512)
                for dc in range(KC):
                    nc.tensor.matmul(
                        s_ps[:, s],
                        q_r[:, dc, pbs],
                        k_r[:, dc, s],
                        start=(dc == 0),
                        stop=False,
                    )
                for dc in range(KC):
                    nc.tensor.matmul(
                        s_ps[:, s],
                        q_8[:, dc, :, pbs],
                        k_8[:, dc, :, s],
                        start=False,
                        stop=(dc == KC - 1),
                        perf_mode=DR,
                    )
                nc.scalar.activation(s_sb[:, s], s_ps[:, s], AF.Identity,
                                     scale=rq_cols[:, pblk:pblk + 1])
                nc.gpsimd.tensor_mul(s_sb[:, s], s_sb[:, s], rk_bcast[:, s])
                nc.gpsimd.tensor_add(s_sb[:, s], s_sb[:, s], pb_sb[:, s])

            # top-16 per row: two rounds of max8 + match_replace
            m8a = work3.tile([128, 8], F32, tag="m8a")
            nc.vector.max(m8a, s_sb)
            sz1 = work3.tile([128, P], F32, tag="sz1")
            nc.vector.match_replace(sz1, in_to_replace=m8a, in_values=s_sb,
                                    imm_value=MINVAL)
            m8b = work3.tile([128, 8], F32, tag="m8b")
            nc.vector.max(m8b, sz1)
            sz2 = work3.tile([128, P], F32, tag="sz2")
            nc.vector.match_replace(sz2, in_to_replace=m8b, in_values=sz1,
                                    imm_value=MINVAL)
            # T = s - sz2: 0 off the top-16, s - MINVAL on it
            t_sb = work3.tile([128, P], F32, tag="t_sb")
            for sl in range(2):
                s = slice(sl * 512, (sl + 1) * 512)
                nc.gpsimd.tensor_sub(t_sb[:, s], s_sb[:, s], sz2[:, s])
            # w = exp((T + MINVAL - m)/TEMP); off-top entries underflow to 0
            ebias = work3.tile([128, 1], F32, tag="ebias")
            nc.vector.tensor_scalar(ebias, m8a[:, 0:1], -MINVAL, -1.0 / TEMP,
                                    op0=OP.add, op1=OP.mult)
            w_sb = wpool.tile([128, P], F16, tag="w_sb")
            den = work3.tile([128, 1], F32, tag="den")
            nc.scalar.activation(w_sb, t_sb, AF.Exp, bias=ebias, scale=1.0 / TEMP,
                                 accum_out=den)
            rden = wpool.tile([128, 1], F32, tag="rden")
            nc.vector.reciprocal(rden, den)
            return w_sb, rden

        def emit_tail(pblk, w_sb, rden):
            """PE tail: transpose w, combine with v, scale, store."""
            pbs = slice(pblk * 128, (pblk + 1) * 128)
            tp_ps = psum3.tile([128, P], F16, tag="tp_ps", name="tp_ps", bufs=2)
            for qc in range(PB):
                nc.tensor.transpose(
                    tp_ps[:, qc * 128:(qc + 1) * 128],
                    w_sb[:, qc * 128:(qc + 1) * 128],
                    ident,
                )
            wT_sb = work3.tile([128, P], F16, tag="wT_sb")
            nc.scalar.activation(wT_sb, tp_ps, AF.Identity)
            o_ps = psum3.tile([128, D], F32, tag="o_ps", name="o_ps", bufs=1)
            for qc in range(PB):
                for sl, s in ((0, slice(0, 512)), (1, slice(512, D))):
                    nc.tensor.matmul(
                        o_ps[:, s],
                        wT_sb[:, qc * 128:(qc + 1) * 128],
                        v_sb[:, qc, s],
                        start=(qc == 0),
                        stop=(qc == PB - 1),
                    )
            out_sb = work3.tile([128, D], F16, tag="out_sb")
            nc.scalar.activation(out_sb, o_ps, AF.Identity, scale=rden)
            nc.sync.dma_start(out[pbs, :], out_sb)

        # software pipeline, 2 blocks deep: block i's PE tail is emitted after
        # block i+2's score matmuls, giving the ~10us top-k/exp vector chain
        # two full block periods of slack before the PE needs its weights.
        pend = []
        for pblk in range(PB):
            pend.append(emit_scores(pblk))
            if pblk >= 2:
                emit_tail(pblk - 2, *pend[pblk - 2])
        emit_tail(PB - 2, *pend[PB - 2])
        emit_tail(PB - 1, *pend[PB - 1])

        wpool.release()
        work3.release()
        psum3.release()
        persist.release()
        consts.release()

    nc.finalize()
    return nc


_PROG_CACHE = {}


def _r12(a):
    """Round fp32 values to 12 explicit mantissa bits (= f32r rounding)."""
    m, e = np.frexp(np.asarray(a, np.float64))
    return ((np.round(m * 4096.0) / 4096.0) * np.exp2(e)).astype(np.float32)


def _w_pack(W):
    """f32r hi part + fp8e5 DoubleRow pair [(Wr/ASC), (Wl*BSC)] for a weight."""
    W = np.asarray(W, dtype=np.float32)
    Wr = _r12(W)
    Wl = (W.astype(np.float64) - Wr).astype(np.float32)
    pair = np.stack([(Wr / ASC).astype(E5), (Wl * BSC).astype(E5)], axis=1)
    return np.ascontiguousarray(Wr), np.ascontiguousarray(pair)


def kernel(**inputs) -> np.ndarray:
    x = np.ascontiguousarray(np.asarray(inputs["x"], dtype=np.float32))
    Wq = np.asarray(inputs["Wq"], dtype=np.float32)
    Wk = np.asarray(inputs["Wk"], dtype=np.float32)
    Wv = np.asarray(inputs["Wv"], dtype=np.float32)
    bq = np.asarray(inputs["bq"], dtype=np.float32)
    bk = np.asarray(inputs["bk"], dtype=np.float32)
    bv = np.asarray(inputs["bv"], dtype=np.float32)
    pos_bias = np.asarray(inputs["pos_bias"], dtype=np.float32)

    with_bias = bool(np.any(bq) or np.any(bk) or np.any(bv))

    # Diagonal is excluded by the reference (set to -1e9 before top-k); any
    # value below every real score gives the identical top-16 and weights.
    pb_adj = np.ascontiguousarray(pos_bias.copy())
    np.fill_diagonal(pb_adj, DIAGVAL)

    if with_bias not in _PROG_CACHE:
        _PROG_CACHE[with_bias] = build_program(with_bias)
    nc = _PROG_CACHE[with_bias]

    Wqr, Wq8 = _w_pack(Wq)
    Wkr, Wk8 = _w_pack(Wk)
    Wvr = np.ascontiguousarray(_r12(Wv))

    in_maps = []
    for b in range(B):
        xTb = np.ascontiguousarray(x[b, 1:, :].T)
        xr = _r12(xTb)
        xl = (xTb.astype(np.float64) - xr).astype(np.float32)
        x8 = np.ascontiguousarray(
            np.stack([(xl * ASC).astype(E5), (xTb / BSC).astype(E5)], axis=1))
        m = {
            "xr": np.ascontiguousarray(xr), "x8": x8,
            "wqr": Wqr, "wq8": Wq8, "wkr": Wkr, "wk8": Wk8, "wvr": Wvr,
            "pb": pb_adj,
        }
        if with_bias:
            m["bqkv"] = np.ascontiguousarray(np.stack([bq, bk, bv])[None])
        in_maps.append(m)

    res = run_bass_kernel_spmd(nc, in_maps, core_ids=list(range(B)))
    return np.stack([res.results[b]["out"] for b in range(B)]).astype(np.float32)


# revision 24
# speedup vs baseline: 1.0074x; 1.0074x over previous
"""Trainium2 Bass kernel for nn_DavidBeansV2 (sparse wormhole attention).

Math (per batch item b, derived from the reference):
  xp = x[b, 1:, :]                                  # [P, D]
  q  = l2norm(xp @ Wq + bq); k = l2norm(xp @ Wk + bk)
  S  = q @ k.T + pos_bias    (diag forced very negative)
  topk16 per row of S/TEMP -> softmax weights w (zero elsewhere)
  v  = xp @ Wv + bv
  out[b] = (w / rowsum(w)) @ v                      # [P, D]
The multihead gather+combine with routes shared across heads is exactly a
row-sparse [P,P] x [P,D] matmul, so we compute it densely on the PE with a
masked-softmax weight matrix.

Sharding: data-parallel over batch B=8 across the 8 NeuronCores.

Precision: top-16 selection needs ~1e-6-accurate scores (the 16/17
boundary gaps concentrate near zero), which rules out any single-pass
matmul.  Instead of the 3x-cost compensated-fp16 scheme, each exact
matmul A@B runs as
    r12(A) @ r12(B)            one fp32r pass   (1 cyc/row, 12-bit operands)
  + [Al' | A'] @ [B' | Bl']    one fp8e5 DoubleRow pass (0.5 cyc/row)
where Al = A - r12(A) and the DoubleRow pair computes Al@B + A@Bl with
host/device-baked power-of-two scale splits so both fp8 products land at
natural scale and accumulate into the same PSUM group.  Total 1.5
cyc/row-equivalent vs 3.0 for the compensated path, with ~1e-6 score
error (fp8 only ever quantizes the ~2^-13-magnitude residual terms).
Normalization is factored out of q/k and applied to the scores
(S = (qraw.kraw) * rq[p] * rk[col] + pb) in fp32 vector ops, keeping all
matmul operands at their raw (losslessly split) values.  V projection is
a single fp32r pass (smooth error only); the combine runs in fp16.
"""

import numpy as np
import ml_dtypes

import concourse.mybir as mybir
import concourse.tile as tile
from concourse import bass_isa
from concourse import bacc
from concourse.bass_utils import run_bass_kernel_spmd
from concourse.masks import make_identity

F32 = mybir.dt.float32
F32R = mybir.dt.float32r
F16 = mybir.dt.float16
F8E5 = mybir.dt.float8e5
AF = mybir.ActivationFunctionType
OP = mybir.AluOpType
DR = mybir.MatmulPerfMode.DoubleRow
E5 = ml_dtypes.float8_e5m2

B, P, D = 8, 1024, 768
TEMP = 0.1
KC = D // 128     # 6 contraction chunks
PB = P // 128     # 8 row blocks
MINVAL = -50.0    # match_replace fill; below any real score, above diag fill
DIAGVAL = -10000.0
ASC = 2.0 ** 6    # xl cross scale: (xl*ASC) @ (Wr/ASC)
BSC = 2.0 ** 10   # Wl cross scale: (x/BSC) @ (Wl*BSC)


def build_program(with_bias: bool):
    nc = bacc.Bacc(
        "TRN2",
        target_bir_lowering=False,
        debug=False,
        enable_asserts=False,
        num_devices=B,
    )
    xr_d = nc.dram_tensor("xr", [D, P], F32R, kind="ExternalInput").ap()
    x8_d = nc.dram_tensor("x8", [D, 2, P], F8E5, kind="ExternalInput").ap()
    wqr_d = nc.dram_tensor("wqr", [D, D], F32R, kind="ExternalInput").ap()
    wq8_d = nc.dram_tensor("wq8", [D, 2, D], F8E5, kind="ExternalInput").ap()
    wkr_d = nc.dram_tensor("wkr", [D, D], F32R, kind="ExternalInput").ap()
    wk8_d = nc.dram_tensor("wk8", [D, 2, D], F8E5, kind="ExternalInput").ap()
    wvr_d = nc.dram_tensor("wvr", [D, D], F32R, kind="ExternalInput").ap()
    pb = nc.dram_tensor("pb", [P, P], F32, kind="ExternalInput").ap()
    if with_bias:
        bqkv = nc.dram_tensor("bqkv", [1, 3, D], F32, kind="ExternalInput").ap()
    out = nc.dram_tensor("out", [P, D], F16, kind="ExternalOutput").ap()

    with tile.TileContext(nc) as tc:
        consts = tc.alloc_tile_pool(name="consts", bufs=1)
        persist = tc.alloc_tile_pool(name="persist", bufs=1)
        wk_pool = tc.alloc_tile_pool(name="wk_pool", bufs=1)
        inp_pool = tc.alloc_tile_pool(name="inp", bufs=1)
        work2 = tc.alloc_tile_pool(name="work2", bufs=1)
        wq_pool = tc.alloc_tile_pool(name="wq_pool", bufs=1)
        psum2 = tc.alloc_tile_pool(name="psum2", bufs=1, space="PSUM")

        ident = consts.tile([128, 128], F16, tag="ident")
        make_identity(nc, ident)
        # warm-up matmuls: keep the PE busy through the initial input-DMA
        # wait so the pstate clock is at full rate when real work starts
        warm_ps = psum2.tile([1, 64], F32, tag="warm_ps")
        for _ in range(60):
            nc.tensor.matmul(warm_ps, ident[:, 0:1], ident[:, 0:64],
                             start=True, stop=True)
        ones_row = consts.tile([1, 512], F32, tag="ones_row")
        nc.vector.memset(ones_row, 1.0)

        # ---- load inputs, chunked so the first matmuls start early ----
        xr_sb = inp_pool.tile([128, KC, P], F32R, tag="xr_sb", name="xr_sb")
        x8_sb = inp_pool.tile([128, KC, 2, P], F8E5, tag="x8_sb", name="x8_sb")
        wqr_sb = wq_pool.tile([128, KC, D], F32R, tag="wqr_sb", name="wqr_sb")
        wq8_sb = wq_pool.tile([128, KC, 2, D], F8E5, tag="wq8_sb", name="wq8_sb")
        wkr_sb = wk_pool.tile([128, KC, D], F32R, tag="wkr_sb", name="wkr_sb")
        wk8_sb = wk_pool.tile([128, KC, 2, D], F8E5, tag="wk8_sb", name="wk8_sb")

        xr_src = xr_d.rearrange("(o p) f -> p o f", p=128)
        x8_src = x8_d.rearrange("(o p) t f -> p o t f", p=128)
        wqr_src = wqr_d.rearrange("(o p) f -> p o f", p=128)
        wq8_src = wq8_d.rearrange("(o p) t f -> p o t f", p=128)
        wkr_src = wkr_d.rearrange("(o p) f -> p o f", p=128)
        wk8_src = wk8_d.rearrange("(o p) t f -> p o t f", p=128)

        # half-granular x loads so the dc-major matmul sweeps chase arrivals
        h0, h1 = slice(0, 512), slice(512, P)
        for dc in range(KC):
            nc.sync.dma_start(xr_sb[:, dc, h0], xr_src[:, dc, h0])
            nc.sync.dma_start(wqr_sb[:, dc, :], wqr_src[:, dc, :])
        for dc in range(KC):
            nc.sync.dma_start(x8_sb[:, dc, :, h0], x8_src[:, dc, :, h0])
            nc.sync.dma_start(wq8_sb[:, dc, :, :], wq8_src[:, dc, :, :])
        for dc in range(KC):
            nc.sync.dma_start(xr_sb[:, dc, h1], xr_src[:, dc, h1])
        for dc in range(KC):
            nc.sync.dma_start(x8_sb[:, dc, :, h1], x8_src[:, dc, :, h1])
        for dc in range(KC):
            nc.sync.dma_start(wkr_sb[:, dc, :], wkr_src[:, dc, :])
        for dc in range(KC):
            nc.sync.dma_start(wk8_sb[:, dc, :, :], wk8_src[:, dc, :, :])
        if with_bias:
            bias_sb = consts.tile([1, 3, D], F32, tag="bias_sb")
            nc.sync.dma_start(bias_sb, bqkv)

        # persistent q/k operands for the scores stage
        q_r = persist.tile([128, KC, P], F32R, tag="q_r", name="q_r")
        k_r = persist.tile([128, KC, P], F32R, tag="k_r", name="k_r")
        q_8 = persist.tile([128, KC, 2, P], F8E5, tag="q_8", name="q_8")
        k_8 = persist.tile([128, KC, 2, P], F8E5, tag="k_8", name="k_8")
        v_sb = persist.tile([128, PB, D], F16, tag="v_sb")
        wpack = {"q": (wqr_sb, wq8_sb), "k": (wkr_sb, wk8_sb)}
        rpack = {"q": (q_r, q_8, 0, 1), "k": (k_r, k_8, 1, 0)}

        # ---- raw q/k projections (f32r + fp8 DoubleRow crosses) + norms ----
        rinv_rows = {}
        for ti, nm in enumerate(("q", "k")):
            wr, w8 = wpack[nm]
            t_r, t_8, l_slot, full_slot = rpack[nm]
            sq_acc = work2.tile([128, P], F32, tag="sq_acc")
            # dc-major sweeps over half-width PSUM groups (one bank per dblk)
            # so each arriving x/w chunk immediately unlocks work on all six
            # output blocks instead of serializing behind the full load.
            for sl in range(2):
                s = slice(sl * 512, (sl + 1) * 512)
                mm = [psum2.tile([128, 512], F32, tag=f"mmh{d}",
                                 name=f"mmh{d}", bufs=1) for d in range(KC)]
                for dc in range(KC):
                    for dblk in range(KC):
                        nc.tensor.matmul(
                            mm[dblk],
                            wr[:, dc, dblk * 128:(dblk + 1) * 128],
                            xr_sb[:, dc, s],
                            start=(dc == 0),
                            stop=False,
                        )
                for dc in range(KC):
                    for dblk in range(KC):
                        nc.tensor.matmul(
                            mm[dblk],
                            w8[:, dc, :, dblk * 128:(dblk + 1) * 128],
                            x8_sb[:, dc, :, s],
                            start=False,
                            stop=(dc == KC - 1) and not with_bias,
                            perf_mode=DR,
                        )
                if with_bias:
                    for dblk in range(KC):
                        nc.tensor.matmul(
                            mm[dblk],
                            bias_sb[:, ti, dblk * 128:(dblk + 1) * 128],
                            ones_row,
                            start=False,
                            stop=True,
                        )
                # split raw projection into f32r hi + fp8 residual/full pair;
                # consumers spread across act/DVE/Pool so no engine queue
                # backs up behind the PSUM rotations
                for dblk in range(KC):
                    nc.scalar.activation(t_r[:, dblk, s], mm[dblk], AF.Identity)
                    nc.scalar.activation(t_8[:, dblk, full_slot, s], mm[dblk],
                                         AF.Identity)
                    nc.vector.tensor_sub(t_8[:, dblk, l_slot, s], mm[dblk],
                                         t_r[:, dblk, s].bitcast(F32))
                    # norm^2 via q_raw*r12(q_raw): 1.3e-6 relative, row-uniform
                    if dblk == 0:
                        nc.vector.tensor_mul(sq_acc[:, s], mm[dblk],
                                             t_r[:, dblk, s].bitcast(F32))
                    else:
                        sq_sb = work2.tile([128, 512], F32, tag="sq_sb")
                        nc.vector.tensor_mul(sq_sb, mm[dblk],
                                             t_r[:, dblk, s].bitcast(F32))
                        nc.gpsimd.tensor_add(sq_acc[:, s], sq_acc[:, s], sq_sb)
            # norm2 = sum over partitions of sq_acc (gpsimd tree reduce),
            # broadcast to all partitions; row 0 feeds the rsqrt chain.
            allr = work2.tile([128, P], F32, tag="sq_sb")
            nc.gpsimd.partition_all_reduce(allr, sq_acc, channels=128,
                                           reduce_op=bass_isa.ReduceOp.add)
            norm2_sb = allr[0:1, :]
            # rinv = norm2^-0.5 via exp(-0.5*ln(norm2)) + one Newton step
            # (ACT spline budgets are too loose for the flip-sensitive top-16)
            ln_row = work2.tile([1, P], F32, tag="ln_row")
            nc.scalar.activation(ln_row, norm2_sb, AF.Ln)
            rinv_row = work2.tile([1, P], F32, tag=f"rinv_{nm}",
                                  name=f"rinv_{nm}")
            nc.scalar.activation(rinv_row, ln_row, AF.Exp, scale=-0.5)
            rr = work2.tile([1, P], F32, tag="rr")
            nc.vector.tensor_mul(rr, rinv_row, rinv_row)
            nc.vector.tensor_mul(rr, rr, norm2_sb)
            nc.vector.tensor_scalar(rr, rr, -0.5, 1.5, op0=OP.mult, op1=OP.add)
            nc.vector.tensor_mul(rinv_row, rinv_row, rr)
            rinv_rows[nm] = rinv_row
            if nm == "q":
                # Wv loads into the arena wq_pool is about to free
                wq_pool.release()
                wv_pool = tc.alloc_tile_pool(name="wv_pool", bufs=1)
                wvr_sb = wv_pool.tile([128, KC, D], F32R, tag="wvr_sb",
                                      name="wvr_sb")
                wvr_src = wvr_d.rearrange("(o p) f -> p o f", p=128)
                for dc in range(KC):
                    nc.sync.dma_start(wvr_sb[:, dc, :], wvr_src[:, dc, :])

        # ---- v projection (single f32r pass, natural [p, d] layout) ----
        for pblk in range(PB):
            vh = [psum2.tile([128, 512], F32, tag=f"mmh{(2 * pblk + i) % KC}",
                             name=f"mmh{(2 * pblk + i) % KC}", bufs=1)
                  for i in range(2)]
            for dc in range(KC):
                for sl, s, n in ((0, slice(0, 512), 512), (1, slice(512, D), 256)):
                    nc.tensor.matmul(
                        vh[sl][:, :n],
                        xr_sb[:, dc, pblk * 128:(pblk + 1) * 128],
                        wvr_sb[:, dc, s],
                        start=(dc == 0),
                        stop=(dc == KC - 1) and not with_bias,
                    )
            if with_bias:
                for sl, s, n in ((0, slice(0, 512), 512), (1, slice(512, D), 256)):
                    nc.tensor.matmul(
                        vh[sl][:, :n],
                        ones_row[:, :128],
                        bias_sb[:, 2, s],
                        start=False,
                        stop=True,
                    )
            nc.scalar.activation(v_sb[:, pblk, 0:512], vh[0], AF.Identity)
            nc.scalar.activation(v_sb[:, pblk, 512:D], vh[1][:, :256], AF.Identity)

        # ---- relayout rinv_q -> per-partition columns, rinv_k -> bcast ----
        # (fp32 matmuls: rinv values must not be re-rounded to f32r)
        rq_cols = persist.tile([128, PB], F32, tag="rq_cols")
        rq_ps = psum2.tile([128, PB], F32, tag="rq_ps", name="rq_ps", bufs=1)
        for j in range(PB):
            nc.tensor.matmul(
                rq_ps[:, j:j + 1],
                rinv_rows["q"][:, j * 128:(j + 1) * 128],
                ones_row[:, 0:1],
                start=True,
                stop=True,
            )
        nc.scalar.activation(rq_cols, rq_ps, AF.Identity)
        rk_bcast = persist.tile([128, P], F32, tag="rk_bcast")
        for sl in range(2):
            s = slice(sl * 512, (sl + 1) * 512)
            bc_ps = psum2.tile([128, 512], F32, tag=f"mmh{sl}",
                               name=f"mmh{sl}", bufs=1)
            nc.tensor.matmul(bc_ps, ones_row[:, :128],
                             rinv_rows["k"][:, s], start=True, stop=True)
            nc.scalar.activation(rk_bcast[:, s], bc_ps, AF.Identity)

        wv_pool.release()
        work2.release()
        inp_pool.release()
        wk_pool.release()
        psum2.release()

        # ---- per row-block: scores, top-16 softmax, combine ----
        work3 = tc.alloc_tile_pool(name="work3", bufs=2)
        wpool = tc.alloc_tile_pool(name="wpool", bufs=3)
        psum3 = tc.alloc_tile_pool(name="psum3", bufs=1, space="PSUM")

        state = {}

        def emit_prep(pblk):
            """S matmuls + fp32 post-scale producing s_sb (no topk yet)."""
            pbs = slice(pblk * 128, (pblk + 1) * 128)
            s_ps = psum3.tile([128, P], F32, tag="s_ps", name="s_ps", bufs=2)
            pb_sb = work3.tile([128, P], F32, tag="pb_sb")
            nc.sync.dma_start(pb_sb, pb[pbs, :])
            s_sb = work3.tile([128, P], F32, tag="s_sb")
            # compute each 512-half to completion so S = Sraw*rq*rk + pb
            # post-processing of half 0 overlaps half 1's matmuls
            for sl in range(2):
                s = slice(sl * 512, (sl + 1) * 512)
                for dc in range(KC):
                    nc.tensor.matmul(
                        s_ps[:, s],
                        q_r[:, dc, pbs],
                        k_r[:, dc, s],
                        start=(dc == 0),
                        stop=False,
                    )
                for dc in range(KC):
                    nc.tensor.matmul(
                        s_ps[:, s],
                        q_8[:, dc, :, pbs],
                        k_8[:, dc, :, s],
                        start=False,
                        stop=(dc == KC - 1),
                        perf_mode=DR,
                    )
                nc.scalar.activation(s_sb[:, s], s_ps[:, s], AF.Identity,
                                     scale=rq_cols[:, pblk:pblk + 1])
                nc.gpsimd.tensor_mul(s_sb[:, s], s_sb[:, s], rk_bcast[:, s])
                nc.gpsimd.tensor_add(s_sb[:, s], s_sb[:, s], pb_sb[:, s])
            state[pblk] = {"s_sb": s_sb}

        def emit_topk(pblk):
            """top-16 + exp for pblk; also the deferred recip of pblk-1."""
            s_sb = state[pblk]["s_sb"]
            # top-16 per row: two rounds of max8 + match_replace
            m8a = work3.tile([128, 8], F32, tag="m8a")
            nc.vector.max(m8a, s_sb)
            sz1 = work3.tile([128, P], F32, tag="sz1")
            nc.vector.match_replace(sz1, in_to_replace=m8a, in_values=s_sb,
                                    imm_value=MINVAL)
            m8b = work3.tile([128, 8], F32, tag="m8b")
            nc.vector.max(m8b, sz1)
            sz2 = work3.tile([128, P], F32, tag="sz2")
            nc.vector.match_replace(sz2, in_to_replace=m8b, in_values=sz1,
                                    imm_value=MINVAL)
            # T = s - sz2: 0 off the top-16, s - MINVAL on it
            t_sb = work3.tile([128, P], F32, tag="t_sb")
            for sl in range(2):
                s = slice(sl * 512, (sl + 1) * 512)
                nc.gpsimd.tensor_sub(t_sb[:, s], s_sb[:, s], sz2[:, s])
            # w = exp((T + MINVAL - m)/TEMP); off-top entries underflow to 0
            ebias = work3.tile([128, 1], F32, tag="ebias")
            nc.vector.tensor_scalar(ebias, m8a[:, 0:1], -MINVAL, -1.0 / TEMP,
                                    op0=OP.add, op1=OP.mult)
            w_sb = wpool.tile([128, P], F16, tag="w_sb")
            den = wpool.tile([128, 1], F32, tag="den")
            nc.scalar.activation(w_sb, t_sb, AF.Exp, bias=ebias, scale=1.0 / TEMP,
                                 accum_out=den)
            state[pblk].update(w_sb=w_sb, den=den)
            if pblk >= 1:
                emit_recip(pblk - 1)

        def emit_recip(pblk):
            rden = wpool.tile([128, 1], F32, tag="rden")
            nc.vector.reciprocal(rden, state[pblk]["den"])
            state[pblk]["rden"] = rden

        def emit_tail(pblk):
            """PE tail: transpose w, combine with v, scale, store."""
            w_sb, rden = state[pblk]["w_sb"], state[pblk]["rden"]
            pbs = slice(pblk * 128, (pblk + 1) * 128)
            tp_ps = psum3.tile([128, P], F16, tag="tp_ps", name="tp_ps", bufs=2)
            for qc in range(PB):
                nc.tensor.transpose(
                    tp_ps[:, qc * 128:(qc + 1) * 128],
                    w_sb[:, qc * 128:(qc + 1) * 128],
                    ident,
                )
            wT_sb = work3.tile([128, P], F16, tag="wT_sb")
            nc.scalar.activation(wT_sb, tp_ps, AF.Identity)
            o_ps = psum3.tile([128, D], F32, tag="o_ps", name="o_ps", bufs=1)
            for qc in range(PB):
                for sl, s in ((0, slice(0, 512)), (1, slice(512, D))):
                    nc.tensor.matmul(
                        o_ps[:, s],
                        wT_sb[:, qc * 128:(qc + 1) * 128],
                        v_sb[:, qc, s],
                        start=(qc == 0),
                        stop=(qc == PB - 1),
                    )
            out_sb = work3.tile([128, D], F16, tag="out_sb")
            nc.scalar.activation(out_sb, o_ps, AF.Identity, scale=rden)
            nc.sync.dma_start(out[pbs, :], out_sb)

        # software pipeline, staged so each engine's in-order queue never
        # carries a cross-block dependency cycle: block i's top-k chain is
        # emitted after block i+1's score prep (so act runs TSP_{i+1} before
        # exp_i), and block i's PE tail after block i+2's prep.
        for pblk in range(PB):
            emit_prep(pblk)
            if pblk >= 1:
                emit_topk(pblk - 1)
            if pblk >= 2:
                emit_tail(pblk - 2)
        emit_topk(PB - 1)
        emit_recip(PB - 1)
        emit_tail(PB - 2)
        emit_tail(PB - 1)

        wpool.release()
        work3.release()
        psum3.release()
        persist.release()
        consts.release()

    nc.finalize()
    return nc


_PROG_CACHE = {}


def _r12(a):
    """Round fp32 values to 12 explicit mantissa bits (= f32r rounding)."""
    m, e = np.frexp(np.asarray(a, np.float64))
    return ((np.round(m * 4096.0) / 4096.0) * np.exp2(e)).astype(np.float32)


def _w_pack(W):
    """f32r hi part + fp8e5 DoubleRow pair [(Wr/ASC), (Wl*BSC)] for a weight."""
    W = np.asarray(W, dtype=np.float32)
    Wr = _r12(W)
    Wl = (W.astype(np.float64) - Wr).astype(np.float32)
    pair = np.stack([(Wr / ASC).astype(E5), (Wl * BSC).astype(E5)], axis=1)
    return np.ascontiguousarray(Wr), np.ascontiguousarray(pair)


def kernel(**inputs) -> np.ndarray:
    x = np.ascontiguousarray(np.asarray(inputs["x"], dtype=np.float32))
    Wq = np.asarray(inputs["Wq"], dtype=np.float32)
    Wk = np.asarray(inputs["Wk"], dtype=np.float32)
    Wv = np.asarray(inputs["Wv"], dtype=np.float32)
    bq = np.asarray(inputs["bq"], dtype=np.float32)
    bk = np.asarray(inputs["bk"], dtype=np.float32)
    bv = np.asarray(inputs["bv"], dtype=np.float32)
    pos_bias = np.asarray(inputs["pos_bias"], dtype=np.float32)

    with_bias = bool(np.any(bq) or np.any(bk) or np.any(bv))

    # Diagonal is excluded by the reference (set to -1e9 before top-k); any
    # value below every real score gives the identical top-16 and weights.
    pb_adj = np.ascontiguousarray(pos_bias.copy())
    np.fill_diagonal(pb_adj, DIAGVAL)

    if with_bias not in _PROG_CACHE:
        _PROG_CACHE[with_bias] = build_program(with_bias)
    nc = _PROG_CACHE[with_bias]

    Wqr, Wq8 = _w_pack(Wq)
    Wkr, Wk8 = _w_pack(Wk)
    Wvr = np.ascontiguousarray(_r12(Wv))

    in_maps = []
    for b in range(B):
        xTb = np.ascontiguousarray(x[b, 1:, :].T)
        xr = _r12(xTb)
        xl = (xTb.astype(np.float64) - xr).astype(np.float32)
        x8 = np.ascontiguousarray(
            np.stack([(xl * ASC).astype(E5), (xTb / BSC).astype(E5)], axis=1))
        m = {
            "xr": np.ascontiguousarray(xr), "x8": x8,
            "wqr": Wqr, "wq8": Wq8, "wkr": Wkr, "wk8": Wk8, "wvr": Wvr,
            "pb": pb_adj,
        }
        if with_bias:
            m["bqkv"] = np.ascontiguousarray(np.stack([bq, bk, bv])[None])
        in_maps.append(m)

    res = run_bass_kernel_spmd(nc, in_maps, core_ids=list(range(B)))
    return np.stack([res.results[b]["out"] for b in range(B)]).astype(np.float32)
